# revision 17
# baseline (speedup 1.0000x reference)
"""Causal self-attention with RoPE (B=2, T=2048, C=1024, H=16, D=64) on 8
Trainium2 NeuronCores.

Sharding: tensor-parallel over heads - each core owns 2 heads (QKV and output
projections sliced on the head axis); the per-core partial outputs (full
[C, B*T] each) are summed on the host.

Rev A (vs fp32r baseline):
  - all matmul operands in bf16 (stationaries + moving): halves LDWEIGHTS
    time and PE/SBUF read bandwidth; PSUM accumulation stays f32
  - weight DMAs split (V first) + bf16 xt so the first matmul starts ~2us
    into the kernel instead of ~20us
  - out-projection DMAs go PSUM -> HBM directly (no engine copy)
  - instruction streams of the four phases are interleaved via generators:
    attn(b0) is co-emitted with qkv(b1) fillers, attn(b1) with out-proj
    fillers, so the PE never idles waiting on exp/RoPE dependency chains

Per-core layout (everything transposed: features on partitions, tokens free):
  xt [128, 4096]      x^T, bf16
  QKV proj            W rows pre-permuted on host into per-kc [V|E|O] groups:
                        E = [q_h0_even(32) | q_h1_even | k_h0_even | k_h1_even]
                        O = same rows, odd dims; V = [v_h0(64) | v_h1(64)]
  RoPE                rotE = E*cos - O*sin, rotO = E*sin + O*cos, scattered
                      into head-contiguous q_t/k_t [128, T] bf16 with layout
                      [h0_e(32) | h0_o(32) | h1_e(32) | h1_o(32)]
  scores^T            S^T[kj,qi] = k_t[h].T @ q_t[h], K=64 per head
  softmax             exp on ScalarE (scale=1/8 folded), causal triangle via
                      gpsimd affine_select, sums via 64 ones-columns in v_all
  PV                  y^T[d,qi] accumulated over kj chunks; v transposed
                      on-chip via PE transpose into v_all [128, 256*16] bf16
  out proj            outT[c,t] partial = woutT . y^T, DMA'd from PSUM

Host gathers the 8 partial outT [1024, 4096] tensors, sums, transposes.
"""

import sys
import types
from collections import deque

import numpy as np

import concourse.bass as bass
import concourse.tile as tile
from concourse import bacc
from concourse import mybir
from concourse.bass_utils import run_bass_kernel_spmd

F32 = mybir.dt.float32
BF16 = mybir.dt.bfloat16

B = 2
T = 2048
C = 1024
D = 64
N_CORES = 8
BT = B * T              # 4096
TC = 512                # token chunk (free dim of most matmuls)
NQI = T // TC           # 4 qi chunks per batch
NKJ = T // 128          # 16 kj chunks per batch
KC = C // 128           # 8 contraction chunks for the projections


def _install_ntff_hook():
    """bass_utils imports antenv.axon_hooks when tracing; this image lacks it.
    Recreate it from the ctypes NTFF driver so trace=True works."""
    if "antenv.axon_hooks" in sys.modules:
        return
    try:
        from trn_agent_boot.trn_boot import _ntff_profile_via_ctypes

        hook = _ntff_profile_via_ctypes("/opt/axon/libaxon_pjrt.so")
    except Exception:
        hook = None
    mod = types.ModuleType("antenv.axon_hooks")
    mod.get_axon_ntff_profile_hook = lambda: hook
    mod.set_axon_ntff_profile_hook = lambda h: None
    sys.modules["antenv.axon_hooks"] = mod


_install_ntff_hook()

# number of qkv(b1) filler units pulled per attn(b0) j-step, and outproj
# filler units per attn(b1) j-step
FILL_B = 3
FILL_C = 1
OUT_DIRECT = False  # PSUM -> DRAM DMA is rejected by bass; copy via DVE


def build_nc():
    nc = bacc.Bacc(None, target_bir_lowering=False, debug=False)

    xt = nc.declare_dram_parameter("xt", [128, (BT // TC) * KC * TC], BF16, isOutput=False)
    # host layout: [V rows (KC*128) | E rows (KC*128) | O rows (KC*128)]
    wqkv = nc.declare_dram_parameter("wqkv", [128, 3 * KC * 128], BF16, isOutput=False)
    wout = nc.declare_dram_parameter("wout", [128, C], BF16, isOutput=False)
    cs = nc.declare_dram_parameter("cs", [128, 2 * T], F32, isOutput=False)
    outT = nc.declare_dram_parameter("outT", [C, BT], BF16, isOutput=True)

    with tile.TileContext(nc) as tc:
        with (
            tc.sbuf_pool(name="statics", bufs=1) as statics,
            tc.sbuf_pool(name="pool_x", bufs=3) as pool_x,
            tc.sbuf_pool(name="pool_rope", bufs=2) as pool_rope,
            tc.sbuf_pool(name="pool_qk", bufs=2) as pool_qk,
            tc.sbuf_pool(name="pool_v", bufs=2) as pool_v,
            tc.sbuf_pool(name="pool_y", bufs=2) as pool_y,
            tc.sbuf_pool(name="pool_vs", bufs=2) as pool_vs,
            tc.sbuf_pool(name="pool_p", bufs=6) as pool_p,
            tc.sbuf_pool(name="pool_o", bufs=3) as pool_o,
            tc.sbuf_pool(name="pool_rb", bufs=2) as pool_rb,
            tc.psum_pool(name="ps_mm", bufs=2) as ps_mm,
            tc.psum_pool(name="ps_st", bufs=2) as ps_st,
            tc.psum_pool(name="ps_y", bufs=2) as ps_y,
        ):
            # weight tiles; V is DMA'd first (first matmuls need it)
            wv_sb = statics.tile([128, KC, 128], BF16)
            we_sb = statics.tile([128, KC, 128], BF16)
            wo_sb = statics.tile([128, KC, 128], BF16)
            wq_v = wqkv.rearrange("p (g kc m) -> p g kc m", g=3, m=128)
            nc.sync.dma_start(out=wv_sb, in_=wq_v[:, 0])
            wout_sb = statics.tile([128, C], BF16)
            cos_sb = statics.tile([128, T], F32)
            sin_sb = statics.tile([128, T], F32)
            statics_emitted = []

            def emit_deferred_statics():
                if statics_emitted:
                    return
                statics_emitted.append(1)
                nc.sync.dma_start(out=we_sb, in_=wq_v[:, 1])
                nc.sync.dma_start(out=wo_sb, in_=wq_v[:, 2])
                nc.sync.dma_start(out=cos_sb, in_=cs[:, 0:T])
                nc.sync.dma_start(out=sin_sb, in_=cs[:, T : 2 * T])
                nc.sync.dma_start(out=wout_sb, in_=wout[:, :])

            state = {}

            def qkv_gen(b):
                q_t = pool_qk.tile([128, T], BF16, tag="q", name=f"q_{b}")
                k_t = pool_qk.tile([128, T], BF16, tag="k", name=f"k_{b}")
                v_all = pool_v.tile([128, 256 * NKJ], BF16, tag="v", name=f"v_{b}")
                state[b] = (q_t, k_t, v_all)

                # ones columns of v_all (64 per head per 256-block)
                vm = v_all.rearrange("p (m c) -> p m c", c=128)
                nc.gpsimd.memset(vm[:, :, 64:128], 1.0)

                for tci in range(4):
                    g = 4 * b + tci
                    xt_sb = pool_x.tile([128, KC, TC], BF16, tag="x", name=f"xt_{g}")
                    nc.sync.dma_start(
                        out=xt_sb,
                        in_=xt[:, g * KC * TC : (g + 1) * KC * TC].rearrange(
                            "p (kc n) -> p kc n", n=TC
                        ),
                    )
                    emit_deferred_statics()
                    yield
                    psv = ps_mm.tile([128, TC], F32, tag="mm", name=f"psv_{g}")
                    for kc in range(KC):
                        nc.tensor.matmul(
                            psv, wv_sb[:, kc], xt_sb[:, kc, :],
                            start=(kc == 0), stop=(kc == KC - 1),
                        )
                        yield
                    v_sb = pool_vs.tile([128, TC], BF16, tag="vs", name=f"vsb_{g}")
                    nc.scalar.activation(
                        out=v_sb, in_=psv, func=mybir.ActivationFunctionType.Copy
                    )
                    yield
                    pse = ps_mm.tile([128, TC], F32, tag="mm", name=f"pse_{g}")
                    for kc in range(KC):
                        nc.tensor.matmul(
                            pse, we_sb[:, kc], xt_sb[:, kc, :],
                            start=(kc == 0), stop=(kc == KC - 1),
                        )
                        yield
                    pso = ps_mm.tile([128, TC], F32, tag="mm", name=f"pso_{g}")
                    for kc in range(KC):
                        nc.tensor.matmul(
                            pso, wo_sb[:, kc], xt_sb[:, kc, :],
                            start=(kc == 0), stop=(kc == KC - 1),
                        )
                        yield

                    # v: transpose 128x128 blocks into v_all via the XBAR
                    # (per-head halves so each DMA dest is contiguous)
                    va = v_all.rearrange("p (n h c) -> p n h c", h=2, c=128)
                    for s in range(4):
                        j = 4 * tci + s
                        for h in range(2):
                            nc.sync.dma_start_transpose(
                                va[:, j, h, 0:64],
                                v_sb[64 * h : 64 * (h + 1), 128 * s : 128 * (s + 1)],
                            )
                    yield

                    # RoPE: full-width products and rotations, then scatter
                    # the 32-row groups into q_t/k_t with SBUF->SBUF DMAs
                    c_sl = cos_sb[:, tci * TC : (tci + 1) * TC]
                    s_sl = sin_sb[:, tci * TC : (tci + 1) * TC]
                    tEC = pool_rope.tile([128, TC], F32, tag="tEC", name=f"tEC_{g}")
                    nc.vector.tensor_mul(out=tEC, in0=pse, in1=c_sl)
                    tES = pool_rope.tile([128, TC], F32, tag="tES", name=f"tES_{g}")
                    nc.vector.tensor_mul(out=tES, in0=pse, in1=s_sl)
                    yield
                    tOS = pool_rope.tile([128, TC], F32, tag="tOS", name=f"tOS_{g}")
                    nc.vector.tensor_mul(out=tOS, in0=pso, in1=s_sl)
                    tOC = pool_rope.tile([128, TC], F32, tag="tOC", name=f"tOC_{g}")
                    nc.vector.tensor_mul(out=tOC, in0=pso, in1=c_sl)
                    yield
                    # rows [q0|q1|k0|k1] (evens in rE, odds in rO)
                    rE = pool_rope.tile([128, TC], BF16, tag="rE", name=f"rE_{g}")
                    nc.vector.tensor_sub(out=rE, in0=tEC, in1=tOS)
                    rO = pool_rope.tile([128, TC], BF16, tag="rO", name=f"rO_{g}")
                    nc.vector.tensor_add(out=rO, in0=tES, in1=tOC)
                    yield

                    sl = slice(tci * TC, (tci + 1) * TC)
                    # q_t/k_t rows [h0e|h0o|h1e|h1o]
                    for h in range(2):
                        nc.sync.dma_start(
                            out=q_t[64 * h : 64 * h + 32, sl],
                            in_=rE[32 * h : 32 * (h + 1)],
                        )
                        nc.sync.dma_start(
                            out=q_t[64 * h + 32 : 64 * h + 64, sl],
                            in_=rO[32 * h : 32 * (h + 1)],
                        )
                        nc.sync.dma_start(
                            out=k_t[64 * h : 64 * h + 32, sl],
                            in_=rE[64 + 32 * h : 64 + 32 * (h + 1)],
                        )
                        nc.sync.dma_start(
                            out=k_t[64 * h + 32 : 64 * h + 64, sl],
                            in_=rO[64 + 32 * h : 64 + 32 * (h + 1)],
                        )
                    yield

            def attn_gen(b):
                q_t, k_t, v_all = state[b]
                y_t = pool_y.tile([128, T], BF16, tag="yt", name=f"y_{b}")
                state[b] = (q_t, k_t, v_all, y_t)
                for i in range(NQI):
                    nj = 4 * i + 4
                    yaccs = {}
                    for h in range(2):
                        yaccs[h] = ps_y.tile(
                            [128, TC], F32, tag="y", name=f"yacc_{b}_{i}_{h}"
                        )

                    def st_of(j):
                        r = j - 4 * i
                        return 128 * r if r > 0 else 0

                    # software-pipelined by one step: PE issues S(j),S(j),
                    # PV(j-1),PV(j-1) back-to-back while exp(j) runs on ACT
                    p_tiles = {}
                    for j in range(nj + 1):
                        if j < nj:
                            st = st_of(j)
                            r = j - 4 * i
                            ksl = slice(128 * j, 128 * (j + 1))
                            qsl = slice(TC * i + st, TC * (i + 1))
                            # both heads' scores into one 2-bank PSUM tile so a
                            # single exp covers the whole j-step
                            ps_s = ps_st.tile(
                                [128, 2 * TC], F32, tag="st", name=f"s_{b}_{i}_{j}",
                            )
                            for h in range(2):
                                hs = slice(64 * h, 64 * (h + 1))
                                nc.tensor.matmul(
                                    ps_s[:, TC * h + st : TC * (h + 1)],
                                    k_t[hs, ksl], q_t[hs, qsl],
                                    start=True, stop=True,
                                )
                            p_sb = pool_p.tile(
                                [128, 2 * TC], BF16, tag="p", name=f"p_{b}_{i}_{j}",
                            )
                            p_tiles[j] = p_sb
                            psv_ = ps_s.rearrange("p (h n) -> p h n", h=2)
                            pbv_ = p_sb.rearrange("p (h n) -> p h n", h=2)
                            nc.scalar.activation(
                                out=pbv_[:, :, st:], in_=psv_[:, :, st:],
                                func=mybir.ActivationFunctionType.Exp,
                                scale=0.125,
                            )
                            if r >= 0:
                                for h in range(2):
                                    nc.gpsimd.affine_select(
                                        out=p_sb[:, TC * h + st : TC * h + st + 128],
                                        in_=p_sb[:, TC * h + st : TC * h + st + 128],
                                        pattern=[[1, 128]],
                                        channel_multiplier=-1,
                                        base=0,
                                        compare_op=mybir.AluOpType.is_ge,
                                        fill=0.0,
                                    )
                        if j >= 1:
                            jp = j - 1
                            st = st_of(jp)
                            p_prev = p_tiles.pop(jp)
                            for h in range(2):
                                nc.tensor.matmul(
                                    yaccs[h][:, st:],
                                    v_all[
                                        :, 256 * jp + 128 * h : 256 * jp + 128 * (h + 1)
                                    ],
                                    p_prev[:, TC * h + st : TC * (h + 1)],
                                    start=(jp == 0),
                                    stop=(jp == nj - 1),
                                )
                        yield ("step", i, j)
                    for h in range(2):
                        rb = pool_rb.tile(
                            [128, TC], F32, tag="rb", name=f"rb_{b}_{i}_{h}"
                        )
                        # full-tile: the custom op mislowers nonzero base
                        # partitions; rows 0:64 are unused garbage recips
                        nc.vector.reciprocal_approx_fast(out=rb, in_=yaccs[h])
                        nc.vector.tensor_mul(
                            out=y_t[64 * h : 64 * (h + 1), TC * i : TC * (i + 1)],
                            in0=yaccs[h][0:64, :],
                            in1=rb[64:128],
                        )
                    yield ("chunk", i)

            def outproj_gen(b, tci):
                y_t = state[b][3]
                g = 4 * b + tci
                for cc in range(KC):
                    ps = ps_mm.tile([128, TC], F32, tag="mm", name=f"op_{g}_{cc}")
                    nc.tensor.matmul(
                        ps,
                        wout_sb[:, 128 * cc : 128 * (cc + 1)],
                        y_t[:, TC * tci : TC * (tci + 1)],
                        start=True,
                        stop=True,
                    )
                    osl = outT[128 * cc : 128 * (cc + 1), g * TC : (g + 1) * TC]
                    o_sb = pool_o.tile([128, TC], BF16, tag="o", name=f"o_{g}_{cc}")
                    if cc % 2 == 0:
                        nc.scalar.activation(
                            out=o_sb, in_=ps, func=mybir.ActivationFunctionType.Copy
                        )
                    else:
                        nc.vector.tensor_copy(out=o_sb, in_=ps)
                    nc.sync.dma_start(out=osl, in_=o_sb)
                    yield

            def drain(gen):
                for _ in gen:
                    pass

            fillers = deque()

            def pull(n):
                while n > 0 and fillers:
                    try:
                        next(fillers[0])
                        n -= 1
                    except StopIteration:
                        fillers.popleft()

            def run_attn(b, fill):
                for ev in attn_gen(b):
                    if ev[0] == "step":
                        pull(fill)
                    elif ev[0] == "chunk":
                        fillers.append(outproj_gen(b, ev[1]))

            # phase A: qkv(b0) alone
            drain(qkv_gen(0))
            # phase B: attn(b0) with qkv(b1) (+ freshly ready outproj) fillers
            qkv1 = qkv_gen(1)
            fillers.append(qkv1)
            run_attn(0, FILL_B)
            # qkv(b1) must be fully emitted before attn(b1) consumers
            if fillers and fillers[0] is qkv1:
                drain(fillers.popleft())
            # phase C: attn(b1) with outproj fillers
            run_attn(1, FILL_C)
            while fillers:
                pull(64)

    nc.compile()
    return nc


_NC_CACHE = None


def _get_nc():
    global _NC_CACHE
    if _NC_CACHE is None:
        _NC_CACHE = build_nc()
    return _NC_CACHE


def _host_prep(x, qkv_w, out_w):
    import ml_dtypes

    x = np.asarray(x, dtype=np.float32)
    qkv_w = np.asarray(qkv_w, dtype=np.float32)
    out_w = np.asarray(out_w, dtype=np.float32)

    # xt[p, ((g*KC)+kc)*TC + n] = x[g*TC + n, kc*128 + p] - one contiguous
    # line per (partition, chunk) for the per-chunk DMA
    xt = np.ascontiguousarray(
        x.reshape(BT // TC, TC, KC, 128)
        .transpose(3, 0, 2, 1)
        .reshape(128, -1)
        .astype(ml_dtypes.bfloat16)
    )

    # rope tables: row p uses frequency index p % 32
    t_idx = np.arange(T, dtype=np.float64)
    inv_freq = 1.0 / (10000.0 ** (np.arange(0, D, 2, dtype=np.float64) / D))  # 32
    ang = np.outer(np.tile(inv_freq, 4), t_idx)  # [128, T]
    cs = np.concatenate(
        [np.cos(ang), np.sin(ang)], axis=1
    ).astype(np.float32)  # [128, 2T]

    in_maps = []
    for core in range(N_CORES):
        h0 = 2 * core
        h1 = h0 + 1
        ev = np.arange(0, D, 2)
        od = np.arange(1, D, 2)
        e_rows = np.concatenate(
            [h0 * D + ev, h1 * D + ev, C + h0 * D + ev, C + h1 * D + ev]
        )
        o_rows = np.concatenate(
            [h0 * D + od, h1 * D + od, C + h0 * D + od, C + h1 * D + od]
        )
        v_rows = np.concatenate(
            [2 * C + h0 * D + np.arange(D), 2 * C + h1 * D + np.arange(D)]
        )
        # group-major layout [V | E | O], each group kc-major
        wqkv_c = np.empty((128, 3 * KC * 128), np.float32)
        for gi, rows in enumerate((v_rows, e_rows, o_rows)):
            w_part = qkv_w[rows]  # [128, C]
            # [p, kc*128 + m] = w_part[m, kc*128 + p]
            blk = w_part.T.reshape(KC, 128, 128).transpose(1, 0, 2).reshape(128, -1)
            wqkv_c[:, gi * KC * 128 : (gi + 1) * KC * 128] = blk
        wqkv_c = np.ascontiguousarray(wqkv_c.astype(ml_dtypes.bfloat16))
        cols = np.concatenate([h0 * D + np.arange(D), h1 * D + np.arange(D)])
        wout_c = np.ascontiguousarray(
            out_w[:, cols].T.astype(ml_dtypes.bfloat16)
        )  # [128, C]
        in_maps.append({"xt": xt, "wqkv": wqkv_c, "wout": wout_c, "cs": cs})
    return in_maps


def _run(in_maps, trace=False):
    nc = _get_nc()
    return run_bass_kernel_spmd(
        nc, in_maps, core_ids=list(range(N_CORES)), trace=trace
    )


def kernel(x, qkv_w, out_w, _trace=False, _results_box=None):
    in_maps = _host_prep(x, qkv_w, out_w)
    res = _run(in_maps, trace=_trace)
    if _results_box is not None:
        _results_box.append(res)
    acc = np.zeros((C, BT), np.float32)
    for r in res.results:
        acc += np.asarray(r["outT"], np.float32)
    out = acc.T.reshape(B, T, C)
    return np.ascontiguousarray(out)


# revision 24
# speedup vs baseline: 1.0866x; 1.0866x over previous
"""Causal self-attention with RoPE (B=2, T=2048, C=1024, H=16, D=64) on 8
Trainium2 NeuronCores.

Sharding: tensor-parallel over heads - each core owns 2 heads (QKV and output
projections sliced on the head axis); the per-core partial outputs (full
[C, B*T] each) are summed on the host.

Rev A (vs fp32r baseline):
  - all matmul operands in bf16 (stationaries + moving): halves LDWEIGHTS
    time and PE/SBUF read bandwidth; PSUM accumulation stays f32
  - weight DMAs split (V first) + bf16 xt so the first matmul starts ~2us
    into the kernel instead of ~20us
  - out-projection DMAs go PSUM -> HBM directly (no engine copy)
  - instruction streams of the four phases are interleaved via generators:
    attn(b0) is co-emitted with qkv(b1) fillers, attn(b1) with out-proj
    fillers, so the PE never idles waiting on exp/RoPE dependency chains

Per-core layout (everything transposed: features on partitions, tokens free):
  xt [128, 4096]      x^T, bf16
  QKV proj            W rows pre-permuted on host into per-kc [V|E|O] groups:
                        E = [q_h0_even(32) | q_h1_even | k_h0_even | k_h1_even]
                        O = same rows, odd dims; V = [v_h0(64) | v_h1(64)]
  RoPE                rotE = E*cos - O*sin, rotO = E*sin + O*cos, scattered
                      into head-contiguous q_t/k_t [128, T] bf16 with layout
                      [h0_e(32) | h0_o(32) | h1_e(32) | h1_o(32)]
  scores^T            S^T[kj,qi] = k_t[h].T @ q_t[h], K=64 per head
  softmax             exp on ScalarE (scale=1/8 folded), causal triangle via
                      gpsimd affine_select, sums via 64 ones-columns in v_all
  PV                  y^T[d,qi] accumulated over kj chunks; v transposed
                      on-chip via PE transpose into v_all [128, 256*16] bf16
  out proj            outT[c,t] partial = woutT . y^T, DMA'd from PSUM

Host gathers the 8 partial outT [1024, 4096] tensors, sums, transposes.
"""

import sys
import types
from collections import deque

import numpy as np

import concourse.bass as bass
import concourse.tile as tile
from concourse import bacc
from concourse import mybir
from concourse.bass_utils import run_bass_kernel_spmd
from concourse.masks import make_identity

F32 = mybir.dt.float32
BF16 = mybir.dt.bfloat16

B = 2
T = 2048
C = 1024
D = 64
N_CORES = 8
BT = B * T              # 4096
TC = 512                # token chunk (free dim of most matmuls)
NQI = T // TC           # 4 qi chunks per batch
NKJ = T // 128          # 16 kj chunks per batch
KC = C // 128           # 8 contraction chunks for the projections


def _install_ntff_hook():
    """bass_utils imports antenv.axon_hooks when tracing; this image lacks it.
    Recreate it from the ctypes NTFF driver so trace=True works."""
    if "antenv.axon_hooks" in sys.modules:
        return
    try:
        from trn_agent_boot.trn_boot import _ntff_profile_via_ctypes

        hook = _ntff_profile_via_ctypes("/opt/axon/libaxon_pjrt.so")
    except Exception:
        hook = None
    mod = types.ModuleType("antenv.axon_hooks")
    mod.get_axon_ntff_profile_hook = lambda: hook
    mod.set_axon_ntff_profile_hook = lambda h: None
    sys.modules["antenv.axon_hooks"] = mod


_install_ntff_hook()

# number of qkv(b1) filler units pulled per attn(b0) j-step, and outproj
# filler units per attn(b1) j-step
FILL_B = 3
FILL_C = 1
PIPE = 2  # S -> PV software-pipeline depth (steps of exp-hiding slack)


def build_nc():
    nc = bacc.Bacc(None, target_bir_lowering=False, debug=False)

    xt = nc.declare_dram_parameter("xt", [128, (BT // TC) * KC * TC], BF16, isOutput=False)
    # host layout: [V rows (KC*128) | E rows (KC*128) | O rows (KC*128)]
    wqkv = nc.declare_dram_parameter("wqkv", [128, 3 * KC * 128], BF16, isOutput=False)
    wout = nc.declare_dram_parameter("wout", [128, C], BF16, isOutput=False)
    cs = nc.declare_dram_parameter("cs", [128, 2 * T], F32, isOutput=False)
    outT = nc.declare_dram_parameter("outT", [C, BT], BF16, isOutput=True)

    with tile.TileContext(nc) as tc:
        with (
            tc.sbuf_pool(name="statics", bufs=1) as statics,
            tc.sbuf_pool(name="pool_x", bufs=3) as pool_x,
            tc.sbuf_pool(name="pool_rope", bufs=2) as pool_rope,
            tc.sbuf_pool(name="pool_qk", bufs=2) as pool_qk,
            tc.sbuf_pool(name="pool_v", bufs=2) as pool_v,
            tc.sbuf_pool(name="pool_y", bufs=2) as pool_y,
            tc.sbuf_pool(name="pool_vs", bufs=2) as pool_vs,
            tc.sbuf_pool(name="pool_p", bufs=8) as pool_p,
            tc.sbuf_pool(name="pool_o", bufs=3) as pool_o,
            tc.sbuf_pool(name="pool_rb", bufs=2) as pool_rb,
            tc.psum_pool(name="ps_mm", bufs=2) as ps_mm,
            tc.psum_pool(name="ps_st", bufs=3) as ps_st,
            tc.psum_pool(name="ps_y", bufs=2) as ps_y,
            tc.psum_pool(name="ps_tr", bufs=1) as ps_tr,
        ):
            ident = statics.tile([128, 128], BF16)
            make_identity(nc, ident)

            # weight tiles; V is DMA'd first (first matmuls need it)
            wv_sb = statics.tile([128, KC, 128], BF16)
            we_sb = statics.tile([128, KC, 128], BF16)
            wo_sb = statics.tile([128, KC, 128], BF16)
            wq_v = wqkv.rearrange("p (g kc m) -> p g kc m", g=3, m=128)
            nc.sync.dma_start(out=wv_sb, in_=wq_v[:, 0])
            wout_sb = statics.tile([128, C], BF16)
            cos_sb = statics.tile([128, T], F32)
            sin_sb = statics.tile([128, T], F32)
            statics_emitted = []

            def emit_deferred_statics():
                if statics_emitted:
                    return
                statics_emitted.append(1)
                nc.sync.dma_start(out=we_sb, in_=wq_v[:, 1])
                nc.sync.dma_start(out=wo_sb, in_=wq_v[:, 2])
                nc.sync.dma_start(out=cos_sb, in_=cs[:, 0:T])
                nc.sync.dma_start(out=sin_sb, in_=cs[:, T : 2 * T])
                nc.sync.dma_start(out=wout_sb, in_=wout[:, :])

            state = {}

            def qkv_gen(b):
                q_t = pool_qk.tile([128, T], BF16, tag="q", name=f"q_{b}")
                k_t = pool_qk.tile([128, T], BF16, tag="k", name=f"k_{b}")
                v_all = pool_v.tile([128, 256 * NKJ], BF16, tag="v", name=f"v_{b}")
                state[b] = (q_t, k_t, v_all)

                # ones columns of v_all (64 per head per 256-block)
                vm = v_all.rearrange("p (m c) -> p m c", c=128)
                nc.gpsimd.memset(vm[:, :, 64:128], 1.0)

                for tci in range(4):
                    g = 4 * b + tci
                    xt_sb = pool_x.tile([128, KC, TC], BF16, tag="x", name=f"xt_{g}")
                    nc.sync.dma_start(
                        out=xt_sb,
                        in_=xt[:, g * KC * TC : (g + 1) * KC * TC].rearrange(
                            "p (kc n) -> p kc n", n=TC
                        ),
                    )
                    emit_deferred_statics()
                    yield
                    psv = ps_mm.tile([128, TC], F32, tag="mm", name=f"psv_{g}")
                    for kc in range(KC):
                        nc.tensor.matmul(
                            psv, wv_sb[:, kc], xt_sb[:, kc, :],
                            start=(kc == 0), stop=(kc == KC - 1),
                        )
                        yield
                    v_sb = pool_vs.tile([128, TC], BF16, tag="vs", name=f"vsb_{g}")
                    nc.scalar.activation(
                        out=v_sb, in_=psv, func=mybir.ActivationFunctionType.Copy
                    )
                    yield
                    pse = ps_mm.tile([128, TC], F32, tag="mm", name=f"pse_{g}")
                    for kc in range(KC):
                        nc.tensor.matmul(
                            pse, we_sb[:, kc], xt_sb[:, kc, :],
                            start=(kc == 0), stop=(kc == KC - 1),
                        )
                        yield
                    pso = ps_mm.tile([128, TC], F32, tag="mm", name=f"pso_{g}")
                    for kc in range(KC):
                        nc.tensor.matmul(
                            pso, wo_sb[:, kc], xt_sb[:, kc, :],
                            start=(kc == 0), stop=(kc == KC - 1),
                        )
                        yield

                    # v: transpose 128x128 blocks into v_all (PE + DVE copy)
                    va = v_all.rearrange("p (n h c) -> p n h c", h=2, c=128)
                    for s in range(4):
                        j = 4 * tci + s
                        tr = ps_tr.tile([128, 128], BF16, tag="tr", name=f"tr_{g}_{s}")
                        nc.tensor.transpose(
                            tr, v_sb[:, 128 * s : 128 * (s + 1)], ident
                        )
                        nc.vector.tensor_copy(
                            out=va[:, j, :, 0:64],
                            in_=tr.rearrange("p (h c) -> p h c", h=2),
                        )
                        yield

                    # RoPE: full-width products and rotations, then scatter
                    # the 32-row groups into q_t/k_t with SBUF->SBUF DMAs
                    c_sl = cos_sb[:, tci * TC : (tci + 1) * TC]
                    s_sl = sin_sb[:, tci * TC : (tci + 1) * TC]
                    tEC = pool_rope.tile([128, TC], F32, tag="tEC", name=f"tEC_{g}")
                    nc.vector.tensor_mul(out=tEC, in0=pse, in1=c_sl)
                    tES = pool_rope.tile([128, TC], F32, tag="tES", name=f"tES_{g}")
                    nc.vector.tensor_mul(out=tES, in0=pse, in1=s_sl)
                    yield
                    tOS = pool_rope.tile([128, TC], F32, tag="tOS", name=f"tOS_{g}")
                    nc.vector.tensor_mul(out=tOS, in0=pso, in1=s_sl)
                    tOC = pool_rope.tile([128, TC], F32, tag="tOC", name=f"tOC_{g}")
                    nc.vector.tensor_mul(out=tOC, in0=pso, in1=c_sl)
                    yield
                    # rows [q0|q1|k0|k1] (evens in rE, odds in rO)
                    rE = pool_rope.tile([128, TC], BF16, tag="rE", name=f"rE_{g}")
                    nc.vector.tensor_sub(out=rE, in0=tEC, in1=tOS)
                    rO = pool_rope.tile([128, TC], BF16, tag="rO", name=f"rO_{g}")
                    nc.vector.tensor_add(out=rO, in0=tES, in1=tOC)
                    yield

                    sl = slice(tci * TC, (tci + 1) * TC)
                    # q_t/k_t rows [h0e|h0o|h1e|h1o]; scatter off the sync
                    # queue (gpsimd-issued direct copies)
                    for h in range(2):
                        nc.gpsimd.dma_start(
                            out=q_t[64 * h : 64 * h + 32, sl],
                            in_=rE[32 * h : 32 * (h + 1)],
                        )
                        nc.gpsimd.dma_start(
                            out=q_t[64 * h + 32 : 64 * h + 64, sl],
                            in_=rO[32 * h : 32 * (h + 1)],
                        )
                        nc.gpsimd.dma_start(
                            out=k_t[64 * h : 64 * h + 32, sl],
                            in_=rE[64 + 32 * h : 64 + 32 * (h + 1)],
                        )
                        nc.gpsimd.dma_start(
                            out=k_t[64 * h + 32 : 64 * h + 64, sl],
                            in_=rO[64 + 32 * h : 64 + 32 * (h + 1)],
                        )
                    yield

            def attn_gen(b):
                q_t, k_t, v_all = state[b]
                y_t = pool_y.tile([128, T], BF16, tag="yt", name=f"y_{b}")
                state[b] = (q_t, k_t, v_all, y_t)
                for i in range(NQI):
                    nj = 4 * i + 4
                    yaccs = {}
                    for h in range(2):
                        yaccs[h] = ps_y.tile(
                            [128, TC], F32, tag="y", name=f"yacc_{b}_{i}_{h}"
                        )

                    def st_of(j):
                        r = j - 4 * i
                        return 128 * r if r > 0 else 0

                    # software-pipelined by one step: PE issues S(j),S(j),
                    # PV(j-1),PV(j-1) back-to-back while exp(j) runs on ACT
                    p_tiles = {}
                    for j in range(nj + PIPE):
                        if j < nj:
                            st = st_of(j)
                            r = j - 4 * i
                            ksl = slice(128 * j, 128 * (j + 1))
                            qsl = slice(TC * i + st, TC * (i + 1))
                            for h in range(2):
                                hs = slice(64 * h, 64 * (h + 1))
                                ps_s = ps_st.tile(
                                    [128, TC], F32, tag="st",
                                    name=f"s_{b}_{i}_{h}_{j}",
                                )
                                nc.tensor.matmul(
                                    ps_s[:, st:], k_t[hs, ksl], q_t[hs, qsl],
                                    start=True, stop=True,
                                )
                                p_sb = pool_p.tile(
                                    [128, TC], BF16, tag="p",
                                    name=f"p_{b}_{i}_{h}_{j}",
                                )
                                p_tiles[(h, j)] = p_sb
                                nc.scalar.activation(
                                    out=p_sb[:, st:], in_=ps_s[:, st:],
                                    func=mybir.ActivationFunctionType.Exp,
                                    scale=0.125,
                                )
                                if r >= 0:
                                    nc.gpsimd.affine_select(
                                        out=p_sb[:, st : st + 128],
                                        in_=p_sb[:, st : st + 128],
                                        pattern=[[1, 128]],
                                        channel_multiplier=-1,
                                        base=0,
                                        compare_op=mybir.AluOpType.is_ge,
                                        fill=0.0,
                                    )
                        if j >= PIPE:
                            jp = j - PIPE
                            st = st_of(jp)
                            for h in range(2):
                                nc.tensor.matmul(
                                    yaccs[h][:, st:],
                                    v_all[
                                        :, 256 * jp + 128 * h : 256 * jp + 128 * (h + 1)
                                    ],
                                    p_tiles.pop((h, jp))[:, st:],
                                    start=(jp == 0),
                                    stop=(jp == nj - 1),
                                )
                        yield ("step", i, j)
                    for h in range(2):
                        rb = pool_rb.tile(
                            [128, TC], F32, tag="rb", name=f"rb_{b}_{i}_{h}"
                        )
                        # full-tile: the custom op mislowers nonzero base
                        # partitions; rows 0:64 are unused garbage recips
                        nc.vector.reciprocal_approx_fast(out=rb, in_=yaccs[h])
                        nc.vector.tensor_mul(
                            out=y_t[64 * h : 64 * (h + 1), TC * i : TC * (i + 1)],
                            in0=yaccs[h][0:64, :],
                            in1=rb[64:128],
                        )
                    yield ("chunk", i)

            def outproj_gen(b, tci):
                y_t = state[b][3]
                g = 4 * b + tci
                for cc in range(KC):
                    ps = ps_mm.tile([128, TC], F32, tag="mm", name=f"op_{g}_{cc}")
                    nc.tensor.matmul(
                        ps,
                        wout_sb[:, 128 * cc : 128 * (cc + 1)],
                        y_t[:, TC * tci : TC * (tci + 1)],
                        start=True,
                        stop=True,
                    )
                    osl = outT[128 * cc : 128 * (cc + 1), g * TC : (g + 1) * TC]
                    o_sb = pool_o.tile([128, TC], BF16, tag="o", name=f"o_{g}_{cc}")
                    if cc % 2 == 0:
                        nc.scalar.activation(
                            out=o_sb, in_=ps, func=mybir.ActivationFunctionType.Copy
                        )
                    else:
                        nc.vector.tensor_copy(out=o_sb, in_=ps)
                    nc.sync.dma_start(out=osl, in_=o_sb)
                    yield

            def drain(gen):
                for _ in gen:
                    pass

            fillers = deque()

            def pull(n):
                while n > 0 and fillers:
                    try:
                        next(fillers[0])
                        n -= 1
                    except StopIteration:
                        fillers.popleft()

            def run_attn(b, fill):
                for ev in attn_gen(b):
                    if ev[0] == "step":
                        pull(fill)
                    elif ev[0] == "chunk":
                        fillers.append(outproj_gen(b, ev[1]))

            # phase A: qkv(b0) alone
            drain(qkv_gen(0))
            # phase B: attn(b0) with qkv(b1) (+ freshly ready outproj) fillers
            qkv1 = qkv_gen(1)
            fillers.append(qkv1)
            run_attn(0, FILL_B)
            # qkv(b1) must be fully emitted before attn(b1) consumers
            if fillers and fillers[0] is qkv1:
                drain(fillers.popleft())
            # phase C: attn(b1) with outproj fillers
            run_attn(1, FILL_C)
            while fillers:
                pull(64)

    nc.compile()
    return nc


_NC_CACHE = None


def _get_nc():
    global _NC_CACHE
    if _NC_CACHE is None:
        _NC_CACHE = build_nc()
    return _NC_CACHE


def _host_prep(x, qkv_w, out_w):
    import ml_dtypes

    x = np.asarray(x, dtype=np.float32)
    qkv_w = np.asarray(qkv_w, dtype=np.float32)
    out_w = np.asarray(out_w, dtype=np.float32)

    # xt[p, ((g*KC)+kc)*TC + n] = x[g*TC + n, kc*128 + p] - one contiguous
    # line per (partition, chunk) for the per-chunk DMA
    xt = np.ascontiguousarray(
        x.reshape(BT // TC, TC, KC, 128)
        .transpose(3, 0, 2, 1)
        .reshape(128, -1)
        .astype(ml_dtypes.bfloat16)
    )

    # rope tables: row p uses frequency index p % 32
    t_idx = np.arange(T, dtype=np.float64)
    inv_freq = 1.0 / (10000.0 ** (np.arange(0, D, 2, dtype=np.float64) / D))  # 32
    ang = np.outer(np.tile(inv_freq, 4), t_idx)  # [128, T]
    cs = np.concatenate(
        [np.cos(ang), np.sin(ang)], axis=1
    ).astype(np.float32)  # [128, 2T]

    in_maps = []
    for core in range(N_CORES):
        h0 = 2 * core
        h1 = h0 + 1
        ev = np.arange(0, D, 2)
        od = np.arange(1, D, 2)
        e_rows = np.concatenate(
            [h0 * D + ev, h1 * D + ev, C + h0 * D + ev, C + h1 * D + ev]
        )
        o_rows = np.concatenate(
            [h0 * D + od, h1 * D + od, C + h0 * D + od, C + h1 * D + od]
        )
        v_rows = np.concatenate(
            [2 * C + h0 * D + np.arange(D), 2 * C + h1 * D + np.arange(D)]
        )
        # group-major layout [V | E | O], each group kc-major
        wqkv_c = np.empty((128, 3 * KC * 128), np.float32)
        for gi, rows in enumerate((v_rows, e_rows, o_rows)):
            w_part = qkv_w[rows]  # [128, C]
            # [p, kc*128 + m] = w_part[m, kc*128 + p]
            blk = w_part.T.reshape(KC, 128, 128).transpose(1, 0, 2).reshape(128, -1)
            wqkv_c[:, gi * KC * 128 : (gi + 1) * KC * 128] = blk
        wqkv_c = np.ascontiguousarray(wqkv_c.astype(ml_dtypes.bfloat16))
        cols = np.concatenate([h0 * D + np.arange(D), h1 * D + np.arange(D)])
        wout_c = np.ascontiguousarray(
            out_w[:, cols].T.astype(ml_dtypes.bfloat16)
        )  # [128, C]
        in_maps.append({"xt": xt, "wqkv": wqkv_c, "wout": wout_c, "cs": cs})
    return in_maps


def _run(in_maps, trace=False):
    nc = _get_nc()
    return run_bass_kernel_spmd(
        nc, in_maps, core_ids=list(range(N_CORES)), trace=trace
    )


def kernel(x, qkv_w, out_w, _trace=False, _results_box=None):
    in_maps = _host_prep(x, qkv_w, out_w)
    res = _run(in_maps, trace=_trace)
    if _results_box is not None:
        _results_box.append(res)
    acc = np.zeros((C, BT), np.float32)
    for r in res.results:
        acc += np.asarray(r["outT"], np.float32)
    out = acc.T.reshape(B, T, C)
    return np.ascontiguousarray(out)


# revision 25
# speedup vs baseline: 1.3619x; 1.2534x over previous
"""Causal self-attention with RoPE (B=2, T=2048, C=1024, H=16, D=64) on 8
Trainium2 NeuronCores.

Sharding: tensor-parallel over heads - each core owns 2 heads (QKV and output
projections sliced on the head axis); the per-core partial outputs (full
[C, B*T] each) are summed on the host.

Rev A (vs fp32r baseline):
  - all matmul operands in bf16 (stationaries + moving): halves LDWEIGHTS
    time and PE/SBUF read bandwidth; PSUM accumulation stays f32
  - weight DMAs split (V first) + bf16 xt so the first matmul starts ~2us
    into the kernel instead of ~20us
  - out-projection DMAs go PSUM -> HBM directly (no engine copy)
  - instruction streams of the four phases are interleaved via generators:
    attn(b0) is co-emitted with qkv(b1) fillers, attn(b1) with out-proj
    fillers, so the PE never idles waiting on exp/RoPE dependency chains

Per-core layout (everything transposed: features on partitions, tokens free):
  xt [128, 4096]      x^T, bf16
  QKV proj            W rows pre-permuted on host into per-kc [V|E|O] groups:
                        E = [q_h0_even(32) | q_h1_even | k_h0_even | k_h1_even]
                        O = same rows, odd dims; V = [v_h0(64) | v_h1(64)]
  RoPE                rotE = E*cos - O*sin, rotO = E*sin + O*cos, scattered
                      into head-contiguous q_t/k_t [128, T] bf16 with layout
                      [h0_e(32) | h0_o(32) | h1_e(32) | h1_o(32)]
  scores^T            S^T[kj,qi] = k_t[h].T @ q_t[h], K=64 per head
  softmax             exp on ScalarE (scale=1/8 folded), causal triangle via
                      gpsimd affine_select, sums via 64 ones-columns in v_all
  PV                  y^T[d,qi] accumulated over kj chunks; v transposed
                      on-chip via PE transpose into v_all [128, 256*16] bf16
  out proj            outT[c,t] partial = woutT . y^T, DMA'd from PSUM

Host gathers the 8 partial outT [1024, 4096] tensors, sums, transposes.
"""

import sys
import types
from collections import deque

import numpy as np

import concourse.bass as bass
import concourse.tile as tile
from concourse import bacc
from concourse import mybir
from concourse.bass_utils import run_bass_kernel_spmd
from concourse.masks import make_identity

F32 = mybir.dt.float32
BF16 = mybir.dt.bfloat16

B = 2
T = 2048
C = 1024
D = 64
N_CORES = 8
BT = B * T              # 4096
TC = 512                # token chunk (free dim of most matmuls)
NQI = T // TC           # 4 qi chunks per batch
NKJ = T // 128          # 16 kj chunks per batch
KC = C // 128           # 8 contraction chunks for the projections


def _install_ntff_hook():
    """bass_utils imports antenv.axon_hooks when tracing; this image lacks it.
    Recreate it from the ctypes NTFF driver so trace=True works."""
    if "antenv.axon_hooks" in sys.modules:
        return
    try:
        from trn_agent_boot.trn_boot import _ntff_profile_via_ctypes

        hook = _ntff_profile_via_ctypes("/opt/axon/libaxon_pjrt.so")
    except Exception:
        hook = None
    mod = types.ModuleType("antenv.axon_hooks")
    mod.get_axon_ntff_profile_hook = lambda: hook
    mod.set_axon_ntff_profile_hook = lambda h: None
    sys.modules["antenv.axon_hooks"] = mod


_install_ntff_hook()

# number of qkv(b1) filler units pulled per attn(b0) j-step, and outproj
# filler units per attn(b1) j-step
FILL_B = 3
FILL_C = 1
PIPE = 2  # S -> PV software-pipeline depth (steps of exp-hiding slack)


def build_nc():
    nc = bacc.Bacc(None, target_bir_lowering=False, debug=False)

    xt = nc.declare_dram_parameter("xt", [128, (BT // TC) * KC * TC], BF16, isOutput=False)
    # host layout: [V rows (KC*128) | E rows (KC*128) | O rows (KC*128)]
    wqkv = nc.declare_dram_parameter("wqkv", [128, 3 * KC * 128], BF16, isOutput=False)
    wout = nc.declare_dram_parameter("wout", [128, C], BF16, isOutput=False)
    cs = nc.declare_dram_parameter("cs", [128, 2 * T], F32, isOutput=False)
    outT = nc.declare_dram_parameter("outT", [C, BT], BF16, isOutput=True)

    with tile.TileContext(nc) as tc:
        with (
            tc.sbuf_pool(name="statics", bufs=1) as statics,
            tc.sbuf_pool(name="pool_x", bufs=3) as pool_x,
            tc.sbuf_pool(name="pool_rope", bufs=2) as pool_rope,
            tc.sbuf_pool(name="pool_qk", bufs=2) as pool_qk,
            tc.sbuf_pool(name="pool_v", bufs=2) as pool_v,
            tc.sbuf_pool(name="pool_y", bufs=2) as pool_y,
            tc.sbuf_pool(name="pool_vs", bufs=2) as pool_vs,
            tc.sbuf_pool(name="pool_p", bufs=8) as pool_p,
            tc.sbuf_pool(name="pool_o", bufs=3) as pool_o,
            tc.sbuf_pool(name="pool_rb", bufs=2) as pool_rb,
            tc.psum_pool(name="ps_mm", bufs=2) as ps_mm,
            tc.psum_pool(name="ps_st", bufs=3) as ps_st,
            tc.psum_pool(name="ps_y", bufs=2) as ps_y,
            tc.psum_pool(name="ps_tr", bufs=1) as ps_tr,
        ):
            ident = statics.tile([128, 128], BF16)
            make_identity(nc, ident)

            # weight tiles; V is DMA'd first (first matmuls need it)
            wv_sb = statics.tile([128, KC, 128], BF16)
            we_sb = statics.tile([128, KC, 128], BF16)
            wo_sb = statics.tile([128, KC, 128], BF16)
            wq_v = wqkv.rearrange("p (g kc m) -> p g kc m", g=3, m=128)
            nc.sync.dma_start(out=wv_sb, in_=wq_v[:, 0])
            wout_sb = statics.tile([128, C], BF16)
            cos_sb = statics.tile([128, T], F32)
            sin_sb = statics.tile([128, T], F32)
            statics_emitted = []

            def emit_deferred_statics():
                if statics_emitted:
                    return
                statics_emitted.append(1)
                nc.sync.dma_start(out=we_sb, in_=wq_v[:, 1])
                nc.sync.dma_start(out=wo_sb, in_=wq_v[:, 2])
                nc.sync.dma_start(out=cos_sb, in_=cs[:, 0:T])
                nc.sync.dma_start(out=sin_sb, in_=cs[:, T : 2 * T])
                nc.sync.dma_start(out=wout_sb, in_=wout[:, :])

            state = {}

            def qkv_gen(b):
                q_t = pool_qk.tile([128, T], BF16, tag="q", name=f"q_{b}")
                k_t = pool_qk.tile([128, T], BF16, tag="k", name=f"k_{b}")
                v_all = pool_v.tile([128, 256 * NKJ], BF16, tag="v", name=f"v_{b}")
                state[b] = (q_t, k_t, v_all)

                # ones columns of v_all (64 per head per 256-block)
                vm = v_all.rearrange("p (m c) -> p m c", c=128)
                nc.gpsimd.memset(vm[:, :, 64:128], 1.0)

                for tci in range(4):
                    g = 4 * b + tci
                    xt_sb = pool_x.tile([128, KC, TC], BF16, tag="x", name=f"xt_{g}")
                    nc.sync.dma_start(
                        out=xt_sb,
                        in_=xt[:, g * KC * TC : (g + 1) * KC * TC].rearrange(
                            "p (kc n) -> p kc n", n=TC
                        ),
                    )
                    emit_deferred_statics()
                    yield
                    psv = ps_mm.tile([128, TC], F32, tag="mm", name=f"psv_{g}")
                    for kc in range(KC):
                        nc.tensor.matmul(
                            psv, wv_sb[:, kc], xt_sb[:, kc, :],
                            start=(kc == 0), stop=(kc == KC - 1),
                        )
                        yield
                    v_sb = pool_vs.tile([128, TC], BF16, tag="vs", name=f"vsb_{g}")
                    nc.scalar.activation(
                        out=v_sb, in_=psv, func=mybir.ActivationFunctionType.Copy
                    )
                    yield
                    pse = ps_mm.tile([128, TC], F32, tag="mm", name=f"pse_{g}")
                    for kc in range(KC):
                        nc.tensor.matmul(
                            pse, we_sb[:, kc], xt_sb[:, kc, :],
                            start=(kc == 0), stop=(kc == KC - 1),
                        )
                        yield
                    pso = ps_mm.tile([128, TC], F32, tag="mm", name=f"pso_{g}")
                    for kc in range(KC):
                        nc.tensor.matmul(
                            pso, wo_sb[:, kc], xt_sb[:, kc, :],
                            start=(kc == 0), stop=(kc == KC - 1),
                        )
                        yield

                    # v: transpose 128x128 blocks into v_all (PE + DVE copy)
                    va = v_all.rearrange("p (n h c) -> p n h c", h=2, c=128)
                    for s in range(4):
                        j = 4 * tci + s
                        tr = ps_tr.tile([128, 128], BF16, tag="tr", name=f"tr_{g}_{s}")
                        nc.tensor.transpose(
                            tr, v_sb[:, 128 * s : 128 * (s + 1)], ident
                        )
                        nc.vector.tensor_copy(
                            out=va[:, j, :, 0:64],
                            in_=tr.rearrange("p (h c) -> p h c", h=2),
                        )
                        yield

                    # RoPE: full-width products and rotations, then scatter
                    # the 32-row groups into q_t/k_t with SBUF->SBUF DMAs
                    c_sl = cos_sb[:, tci * TC : (tci + 1) * TC]
                    s_sl = sin_sb[:, tci * TC : (tci + 1) * TC]
                    tEC = pool_rope.tile([128, TC], F32, tag="tEC", name=f"tEC_{g}")
                    nc.vector.tensor_mul(out=tEC, in0=pse, in1=c_sl)
                    tES = pool_rope.tile([128, TC], F32, tag="tES", name=f"tES_{g}")
                    nc.vector.tensor_mul(out=tES, in0=pse, in1=s_sl)
                    yield
                    tOS = pool_rope.tile([128, TC], F32, tag="tOS", name=f"tOS_{g}")
                    nc.vector.tensor_mul(out=tOS, in0=pso, in1=s_sl)
                    tOC = pool_rope.tile([128, TC], F32, tag="tOC", name=f"tOC_{g}")
                    nc.vector.tensor_mul(out=tOC, in0=pso, in1=c_sl)
                    yield
                    # rows [q0|q1|k0|k1] (evens in rE, odds in rO)
                    rE = pool_rope.tile([128, TC], BF16, tag="rE", name=f"rE_{g}")
                    nc.vector.tensor_sub(out=rE, in0=tEC, in1=tOS)
                    rO = pool_rope.tile([128, TC], BF16, tag="rO", name=f"rO_{g}")
                    nc.vector.tensor_add(out=rO, in0=tES, in1=tOC)
                    yield

                    sl = slice(tci * TC, (tci + 1) * TC)
                    # q_t/k_t rows [h0e|h0o|h1e|h1o]; scatter off the sync
                    # queue (gpsimd-issued direct copies)
                    for h in range(2):
                        nc.gpsimd.dma_start(
                            out=q_t[64 * h : 64 * h + 32, sl],
                            in_=rE[32 * h : 32 * (h + 1)],
                        )
                        nc.gpsimd.dma_start(
                            out=q_t[64 * h + 32 : 64 * h + 64, sl],
                            in_=rO[32 * h : 32 * (h + 1)],
                        )
                        nc.gpsimd.dma_start(
                            out=k_t[64 * h : 64 * h + 32, sl],
                            in_=rE[64 + 32 * h : 64 + 32 * (h + 1)],
                        )
                        nc.gpsimd.dma_start(
                            out=k_t[64 * h + 32 : 64 * h + 64, sl],
                            in_=rO[64 + 32 * h : 64 + 32 * (h + 1)],
                        )
                    yield

            def attn_gen(b):
                q_t, k_t, v_all = state[b]
                y_t = pool_y.tile([128, T], BF16, tag="yt", name=f"y_{b}")
                state[b] = (q_t, k_t, v_all, y_t)
                for i in range(NQI):
                    nj = 4 * i + 4
                    yaccs = {}
                    for h in range(2):
                        yaccs[h] = ps_y.tile(
                            [128, TC], F32, tag="y", name=f"yacc_{b}_{i}_{h}"
                        )

                    def st_of(j):
                        r = j - 4 * i
                        return 128 * r if r > 0 else 0

                    # software-pipelined by one step: PE issues S(j),S(j),
                    # PV(j-1),PV(j-1) back-to-back while exp(j) runs on ACT
                    p_tiles = {}
                    for j in range(nj + PIPE):
                        if j < nj:
                            st = st_of(j)
                            r = j - 4 * i
                            ksl = slice(128 * j, 128 * (j + 1))
                            qsl = slice(TC * i + st, TC * (i + 1))
                            for h in range(2):
                                hs = slice(64 * h, 64 * (h + 1))
                                ps_s = ps_st.tile(
                                    [128, TC], F32, tag="st",
                                    name=f"s_{b}_{i}_{h}_{j}",
                                )
                                nc.tensor.matmul(
                                    ps_s[:, st:], k_t[hs, ksl], q_t[hs, qsl],
                                    start=True, stop=True,
                                )
                                p_sb = pool_p.tile(
                                    [128, TC], BF16, tag="p",
                                    name=f"p_{b}_{i}_{h}_{j}",
                                )
                                p_tiles[(h, j)] = p_sb
                                nc.scalar.activation(
                                    out=p_sb[:, st:], in_=ps_s[:, st:],
                                    func=mybir.ActivationFunctionType.Exp,
                                    scale=0.125,
                                )
                                if r >= 0:
                                    nc.gpsimd.affine_select(
                                        out=p_sb[:, st : st + 128],
                                        in_=p_sb[:, st : st + 128],
                                        pattern=[[1, 128]],
                                        channel_multiplier=-1,
                                        base=0,
                                        compare_op=mybir.AluOpType.is_ge,
                                        fill=0.0,
                                    )
                        if j >= PIPE:
                            jp = j - PIPE
                            st = st_of(jp)
                            for h in range(2):
                                nc.tensor.matmul(
                                    yaccs[h][:, st:],
                                    v_all[
                                        :, 256 * jp + 128 * h : 256 * jp + 128 * (h + 1)
                                    ],
                                    p_tiles.pop((h, jp))[:, st:],
                                    start=(jp == 0),
                                    stop=(jp == nj - 1),
                                )
                        yield ("step", i, j)
                    for h in range(2):
                        rb = pool_rb.tile(
                            [128, TC], F32, tag="rb", name=f"rb_{b}_{i}_{h}"
                        )
                        # full-tile: the custom op mislowers nonzero base
                        # partitions; rows 0:64 are unused garbage recips
                        nc.vector.reciprocal_approx_fast(out=rb, in_=yaccs[h])
                        nc.vector.tensor_mul(
                            out=y_t[64 * h : 64 * (h + 1), TC * i : TC * (i + 1)],
                            in0=yaccs[h][0:64, :],
                            in1=rb[64:128],
                        )
                    yield ("chunk", i)

            def outproj_gen(b, tci):
                y_t = state[b][3]
                g = 4 * b + tci
                for cc in range(KC):
                    ps = ps_mm.tile([128, TC], F32, tag="mm", name=f"op_{g}_{cc}")
                    nc.tensor.matmul(
                        ps,
                        wout_sb[:, 128 * cc : 128 * (cc + 1)],
                        y_t[:, TC * tci : TC * (tci + 1)],
                        start=True,
                        stop=True,
                    )
                    osl = outT[128 * cc : 128 * (cc + 1), g * TC : (g + 1) * TC]
                    o_sb = pool_o.tile([128, TC], BF16, tag="o", name=f"o_{g}_{cc}")
                    nc.vector.tensor_copy(out=o_sb, in_=ps)
                    nc.sync.dma_start(out=osl, in_=o_sb)
                    yield

            def drain(gen):
                for _ in gen:
                    pass

            fillers = deque()

            def pull(n):
                while n > 0 and fillers:
                    try:
                        next(fillers[0])
                        n -= 1
                    except StopIteration:
                        fillers.popleft()

            def run_attn(b, fill):
                for ev in attn_gen(b):
                    if ev[0] == "step":
                        pull(fill)
                    elif ev[0] == "chunk":
                        fillers.append(outproj_gen(b, ev[1]))

            # phase A: qkv(b0) alone
            drain(qkv_gen(0))
            # phase B: attn(b0) with qkv(b1) (+ freshly ready outproj) fillers
            qkv1 = qkv_gen(1)
            fillers.append(qkv1)
            run_attn(0, FILL_B)
            # qkv(b1) must be fully emitted before attn(b1) consumers
            if fillers and fillers[0] is qkv1:
                drain(fillers.popleft())
            # phase C: attn(b1) with outproj fillers
            run_attn(1, FILL_C)
            while fillers:
                pull(64)

    nc.compile()
    return nc


_NC_CACHE = None


def _get_nc():
    global _NC_CACHE
    if _NC_CACHE is None:
        _NC_CACHE = build_nc()
    return _NC_CACHE


def _host_prep(x, qkv_w, out_w):
    import ml_dtypes

    x = np.asarray(x, dtype=np.float32)
    qkv_w = np.asarray(qkv_w, dtype=np.float32)
    out_w = np.asarray(out_w, dtype=np.float32)

    # xt[p, ((g*KC)+kc)*TC + n] = x[g*TC + n, kc*128 + p] - one contiguous
    # line per (partition, chunk) for the per-chunk DMA
    xt = np.ascontiguousarray(
        x.reshape(BT // TC, TC, KC, 128)
        .transpose(3, 0, 2, 1)
        .reshape(128, -1)
        .astype(ml_dtypes.bfloat16)
    )

    # rope tables: row p uses frequency index p % 32
    t_idx = np.arange(T, dtype=np.float64)
    inv_freq = 1.0 / (10000.0 ** (np.arange(0, D, 2, dtype=np.float64) / D))  # 32
    ang = np.outer(np.tile(inv_freq, 4), t_idx)  # [128, T]
    cs = np.concatenate(
        [np.cos(ang), np.sin(ang)], axis=1
    ).astype(np.float32)  # [128, 2T]

    in_maps = []
    for core in range(N_CORES):
        h0 = 2 * core
        h1 = h0 + 1
        ev = np.arange(0, D, 2)
        od = np.arange(1, D, 2)
        e_rows = np.concatenate(
            [h0 * D + ev, h1 * D + ev, C + h0 * D + ev, C + h1 * D + ev]
        )
        o_rows = np.concatenate(
            [h0 * D + od, h1 * D + od, C + h0 * D + od, C + h1 * D + od]
        )
        v_rows = np.concatenate(
            [2 * C + h0 * D + np.arange(D), 2 * C + h1 * D + np.arange(D)]
        )
        # group-major layout [V | E | O], each group kc-major
        wqkv_c = np.empty((128, 3 * KC * 128), np.float32)
        for gi, rows in enumerate((v_rows, e_rows, o_rows)):
            w_part = qkv_w[rows]  # [128, C]
            # [p, kc*128 + m] = w_part[m, kc*128 + p]
            blk = w_part.T.reshape(KC, 128, 128).transpose(1, 0, 2).reshape(128, -1)
            wqkv_c[:, gi * KC * 128 : (gi + 1) * KC * 128] = blk
        wqkv_c = np.ascontiguousarray(wqkv_c.astype(ml_dtypes.bfloat16))
        cols = np.concatenate([h0 * D + np.arange(D), h1 * D + np.arange(D)])
        wout_c = np.ascontiguousarray(
            out_w[:, cols].T.astype(ml_dtypes.bfloat16)
        )  # [128, C]
        in_maps.append({"xt": xt, "wqkv": wqkv_c, "wout": wout_c, "cs": cs})
    return in_maps


def _run(in_maps, trace=False):
    nc = _get_nc()
    return run_bass_kernel_spmd(
        nc, in_maps, core_ids=list(range(N_CORES)), trace=trace
    )


def kernel(x, qkv_w, out_w, _trace=False, _results_box=None):
    in_maps = _host_prep(x, qkv_w, out_w)
    res = _run(in_maps, trace=_trace)
    if _results_box is not None:
        _results_box.append(res)
    acc = np.zeros((C, BT), np.float32)
    for r in res.results:
        acc += np.asarray(r["outT"], np.float32)
    out = acc.T.reshape(B, T, C)
    return np.ascontiguousarray(out)
